# revision 27
# baseline (speedup 1.0000x reference)
"""Trainium2 Bass kernel for nn_EATN (dense_transformer) — v3.

Data-parallel over batch: 32 images -> 8 NeuronCores x 4 images.
Layout: channel-major [C<=144 partitions, N=1024 free] throughout.

v3 vs v2 (scheduler-friendly restructure):
- per-branch PSUM pools (vps/dps/tps/eps) + per-branch SBUF tags so the
  tile list scheduler can overlap the channel branch (LN/GISSA/MLP
  chains) with the spatial branch (convs/SWSA) and the next image's
  stem convs; issue order interleaves the branches as a priority hint.
- all weights packed into one bf16 [128, 8624] tile (2 DMAs) + one f32
  [128, 301] tile; image-0 input DMA issued first so compute starts
  early instead of behind ~60 weight DMAs.
- bf16 stationaries (half LDWEIGHTS) and bf16 movers for every matmul
  with free < 256 (fp32r pays 4x cycles there).
- sqrt/rsqrt via Ln+Exp so the natural_log_exp act table stays
  resident; only Gelu switches tables (2 loads/image instead of ~7).
- softmax stabilization dropped: max exp argument ~25 so unnormalized
  exp stays far below f32/bf16 range; normalization cancels offsets.
- conv-input tiles (xpad/xsl/xsh/xsh9) in bf16 (residuals stay f32r).
- some elementwise adds/muls moved to the idle GPSIMD (Pool) engine.
"""

import sys

if "/opt/trn_rl_repo" not in sys.path:
    sys.path.insert(0, "/opt/trn_rl_repo")

import numpy as np

import concourse.bass as bass
import concourse.tile as tile
from concourse import bacc, mybir
from concourse import bass_utils

F32 = mybir.dt.float32
F32R = mybir.dt.float32r
BF16 = mybir.dt.bfloat16
U32 = mybir.dt.uint32
AF = mybir.ActivationFunctionType
OP = mybir.AluOpType

NCORES = 8
BPC = 4          # images per core
C = 128
N = 1024
PW = 34          # padded width
PN = PW * PW     # 1156
EPS = 1e-5
S_HD = 8.0 ** -0.5
S_HEADS = 16.0 ** -0.5
S_C = 128.0 ** -0.5
TAPS = [dy * PW + dx for dy in range(3) for dx in range(3)]  # 0..70

_COMPILED = None

# bf16 weight-pack column layout (chunk A = stem weights, chunk B = rest).
# Matmult operands must be both-32-bit or both-16-bit (NCC_IBIR034), so
# weights pairing with f32r streams live in the separate f32r pack below.
_L16 = [
    ("wssfe_lo", 1296), ("wssfe_h9", 144), ("wssfe_h8", 144),
    ("wcc_lo", 128), ("wcc_hi", 128),
    ("wcs_lo", 1152), ("wcs_h9", 128), ("wcs_h8", 128),     # A = 3248
    ("wlfe0", 1152), ("wlfe1", 1152),
    ("Ws0piq", 128), ("Ws0piv", 128), ("Ws0po", 128),
    ("Ws1pi", 128), ("Ws1po", 128),
    ("P_b", 128), ("ident", 128),
    ("W1bd", 384), ("W2bd", 384), ("W1m", 512), ("W2m", 512),
    ("P_a", 128), ("sel8", 1024),
]
_A16 = 3248
_N16 = sum(w for _, w in _L16)

_LR = [("meanmat", 128)]
_NR = sum(w for _, w in _LR)

_L32 = [("mask1", 128), ("mask2", 128), ("fcw", 16), ("b1m", 4),
        ("lamv", 2)] + [(nm, 1) for nm in (
            "gssfe_lo", "bssfe_lo", "gssfe_hi", "bssfe_hi",
            "gcc", "bcc", "gcs", "bcs", "glfe0", "blfe0", "glfe1", "blfe1",
            "ln1g", "ln1b", "gbng", "gbnb", "b1v", "b2v", "b2m",
            "bs0piq", "bs0po", "bs1po", "epsb")]
_N32 = sum(w for _, w in _L32)           # 301


def _cols(layout):
    off, d = 0, {}
    for nm, w in layout:
        d[nm] = (off, w)
        off += w
    return d


_C16 = _cols(_L16)
_C32 = _cols(_L32)
_CR = _cols(_LR)


def _rr(a):
    """Host-side RNE rounding to the fp32r grid (drop low 12 mantissa bits)."""
    a = np.ascontiguousarray(a, np.float32)
    b = a.view(np.uint32).astype(np.uint64)
    add = np.uint64((1 << 11) - 1) + ((b >> np.uint64(12)) & np.uint64(1))
    out = ((b + add) >> np.uint64(12) << np.uint64(12)).astype(np.uint32)
    return out.view(np.float32).copy()


def _win(t, p, off, rows=16, cols=32, rs=PW):
    """Strided [p, rows, cols] window into a flat [p, 1156] padded tile."""
    a = t[0:p, :]
    return bass.AP(tensor=a.tensor, offset=a.offset + off,
                   ap=[a.ap[0], [rs, rows], [1, cols]])


def _build():
    # measured Pool elementwise throughput is ~1.3 ns/elem (not the spec's
    # 0.83); calibrate so the list scheduler orders around gpsimd correctly
    from concourse import hw_specs
    hw_specs.TRN2Spec.CYCLE_T[mybir.EngineType.Pool] = 1.31
    nc = bacc.Bacc("TRN2", target_bir_lowering=False, debug=False,
                   num_devices=NCORES)

    d = {
        "xpad": nc.dram_tensor("xpad", [BPC, 144, PN], BF16,
                               kind="ExternalInput"),
        "xpad9": nc.dram_tensor("xpad9", [BPC, 128, PN], BF16,
                                kind="ExternalInput"),
        "pk16": nc.dram_tensor("pk16", [128, _N16], BF16,
                               kind="ExternalInput"),
        "pk32": nc.dram_tensor("pk32", [128, _N32], F32,
                               kind="ExternalInput"),
        "pkr": nc.dram_tensor("pkr", [128, _NR], F32R,
                              kind="ExternalInput"),
        "bbr": nc.dram_tensor("bbr", [1, 768], F32, kind="ExternalInput"),
    }
    d_out = nc.dram_tensor("out", [BPC, 16], F32, kind="ExternalOutput")

    with tile.TileContext(nc) as tc:
        wp = tc.alloc_tile_pool(name="wp", bufs=1)
        tp = tc.alloc_tile_pool(name="tp", bufs=1)
        dw = tc.alloc_tile_pool(name="dw", bufs=1)
        scr = tc.alloc_tile_pool(name="scr", bufs=1)
        sm = tc.alloc_tile_pool(name="sm", bufs=1)
        psV = tc.alloc_tile_pool(name="psV", bufs=2, space="PSUM")
        psD = tc.alloc_tile_pool(name="psD", bufs=2, space="PSUM")
        psT = tc.alloc_tile_pool(name="psT", bufs=1, space="PSUM")
        psE = tc.alloc_tile_pool(name="psE", bufs=3, space="PSUM")

        # ---- persistent tiles; image-0 input DMA first, then weights
        XPL = [wp.tile([128, PN], BF16, tag=f"xpl{k}", name=f"xpl{k}")
               for k in range(2)]
        XPH = [wp.tile([16, PN], BF16, tag=f"xph{k}", name=f"xph{k}")
               for k in range(2)]
        XPH9 = [wp.tile([128, PN], BF16, tag=f"xph9{k}", name=f"xph9{k}")
                for k in range(2)]
        PK16 = wp.tile([128, _N16], BF16, tag="pk16", name="PK16")
        PK32 = wp.tile([128, _N32], F32, tag="pk32", name="PK32")
        PKR = wp.tile([128, _NR], F32R, tag="pkr", name="PKR")
        BBR = wp.tile([128, 768], F32, tag="bbr", name="BBR")

        def load_img(i):
            nc.sync.dma_start(XPL[i % 2][:], d["xpad"].ap()[i, 0:128])
            nc.sync.dma_start(XPH[i % 2][:], d["xpad"].ap()[i, 128:144])
            nc.sync.dma_start(XPH9[i % 2][:], d["xpad9"].ap()[i])

        load_img(0)
        nc.sync.dma_start(PK16[:, 0:_A16], d["pk16"].ap()[:, 0:_A16])
        nc.sync.dma_start(PK32[:], d["pk32"].ap())
        nc.sync.dma_start(BBR[:], d["bbr"].ap().to_broadcast([128, 768]))
        nc.sync.dma_start(PK16[:, _A16:_N16], d["pk16"].ap()[:, _A16:_N16])
        nc.sync.dma_start(PKR[:], d["pkr"].ap())

        def w16(nm, a=None, b=None, p=128):
            c0, w = _C16[nm]
            if a is None:
                a, b = 0, w
            return PK16[0:p, c0 + a:c0 + b]

        def w32(nm, p=128):
            c0, w = _C32[nm]
            return PK32[0:p, c0:c0 + w]

        def wr(nm, a=None, b=None, p=128):
            c0, w = _CR[nm]
            if a is None:
                a, b = 0, w
            return PKR[0:p, c0 + a:c0 + b]

        MEAN = wr("meanmat")
        IDENT = w16("ident")
        EPSB = w32("epsb")

        XSL = wp.tile([128, PN], BF16, tag="xsl", name="XSL")
        XSH = wp.tile([16, PN], BF16, tag="xsh", name="XSH")
        XSH9 = wp.tile([128, PN], BF16, tag="xsh9", name="XSH9")
        XP = [wp.tile([128, PN], BF16, tag=f"xp{k}", name=f"xp{k}")
              for k in range(3)]
        for t in (XSL, XSH, XSH9):
            nc.vector.memset(t[:], 0.0)
        for t in XP:
            nc.vector.memset(t[:], 0.0)
        pooled = wp.tile([128, BPC], F32, tag="pooled", name="pooled")

        def ln_pair(src, sq, fc):
            """mean/meansq matmuls + rstd for 512-chunk fc. Returns
            (mps[128,512] psum, rstd[128,512] sbuf)."""
            sl = slice(fc * 512, fc * 512 + 512)
            mps = psT.tile([128, 512], F32, tag="tps", name="mps")
            nc.tensor.matmul(mps[:], MEAN, src[:, sl], start=True, stop=True)
            sps = psD.tile([128, 512], F32, tag="dps", name="sps")
            nc.tensor.matmul(sps[:], MEAN, sq[:, sl], start=True, stop=True)
            b = scr.tile([128, 512], F32, tag="hd", bufs=3, name="b")
            nc.scalar.activation(b[:], mps[:], AF.Square)
            v = scr.tile([128, 512], F32, tag="hd", bufs=3, name="v")
            nc.vector.tensor_sub(v[:], sps[:], b[:])
            l = scr.tile([128, 512], F32, tag="hd", bufs=3, name="l")
            nc.scalar.activation(l[:], v[:], AF.Ln, bias=EPSB)
            rstd = scr.tile([128, 512], F32, tag="hd", bufs=3, name="rstd")
            nc.scalar.activation(rstd[:], l[:], AF.Exp, scale=-0.5)
            return mps, rstd

        def ss_attn(S_ps, mask, scale):
            """softmax(sign*sqrt(|scale*S|+eps)); A [128,128] bf16 sbuf."""
            r1 = sm.tile([128, 128], F32, tag="ssa", bufs=4, name="r1")
            nc.scalar.activation(r1[:], S_ps[:], AF.Abs, scale=scale)
            l = sm.tile([128, 128], F32, tag="ssa", bufs=4, name="l")
            nc.scalar.activation(l[:], r1[:], AF.Ln, bias=EPSB)
            r2 = sm.tile([128, 128], F32, tag="ssa", bufs=4, name="r2")
            nc.scalar.activation(r2[:], l[:], AF.Exp, scale=0.5)
            sb = sm.tile([128, 128], F32, tag="ssa", bufs=4, name="sb")
            nc.vector.tensor_single_scalar(sb[:].bitcast(U32),
                                           S_ps[:].bitcast(U32),
                                           0x80000000, op=OP.bitwise_and)
            g = sm.tile([128, 128], F32, tag="ssa", bufs=4, name="g")
            nc.vector.tensor_tensor(g[:].bitcast(U32), r2[:].bitcast(U32),
                                    sb[:].bitcast(U32), op=OP.bitwise_or)
            e = sm.tile([128, 128], F32, tag="ssa", bufs=4, name="e")
            nc.scalar.activation(e[:], g[:], AF.Exp)
            em = sm.tile([128, 128], F32, tag="ssa", bufs=4, name="em")
            rs = sm.tile([128, 1], F32, tag="sss", bufs=4, name="rs")
            nc.vector.scalar_tensor_tensor(em[:], e[:], 1.0, mask,
                                           op0=OP.mult, op1=OP.mult,
                                           accum_out=rs[:, 0:1])
            rr = sm.tile([128, 1], F32, tag="sss", bufs=4, name="rr")
            nc.vector.reciprocal(rr[:], rs[:])
            A = sm.tile([128, 128], BF16, tag="ssA", bufs=2, name="A")
            nc.vector.tensor_scalar_mul(A[:], em[:], rr[:, 0:1])
            return A

        for i in range(BPC):
            xpl, xph, xph9 = XPL[i % 2], XPH[i % 2], XPH9[i % 2]
            xp1, xp3 = XP[(2 * i) % 3], XP[(2 * i + 1) % 3]

            # ================= C: stem (ssfe, cc, cs) =================
            for fc in range(2):
                fb = fc * 544
                ps = psV.tile([128, 512], F32, tag="vps", name="ps_sl")
                for tap in range(9):
                    nc.tensor.matmul(ps[:], w16("wssfe_lo", tap * 144,
                                                tap * 144 + 128),
                                     _win(xpl, 128, TAPS[tap] + fb),
                                     start=(tap == 0), stop=False)
                nc.tensor.matmul(ps[:], w16("wssfe_h9", 0, 128),
                                 _win(xph9, 128, fb), start=False, stop=False)
                nc.tensor.matmul(ps[:], w16("wssfe_h8", 0, 128, p=16),
                                 _win(xph, 16, 70 + fb),
                                 start=False, stop=True)
                nc.scalar.activation(_win(XSL, 128, 35 + fb), ps[:], AF.Relu,
                                     bias=w32("bssfe_lo"),
                                     scale=w32("gssfe_lo"))
            for fc in range(2):
                fb = fc * 544
                ps2 = psV.tile([128, 512], F32, tag="vps", name="ps_sh")
                for tap in range(9):
                    nc.tensor.matmul(ps2[0:16, :],
                                     w16("wssfe_lo", tap * 144 + 128,
                                         tap * 144 + 144),
                                     _win(xpl, 128, TAPS[tap] + fb),
                                     start=(tap == 0), stop=False)
                nc.tensor.matmul(ps2[0:16, :], w16("wssfe_h9", 128, 144),
                                 _win(xph9, 128, fb), start=False, stop=False)
                nc.tensor.matmul(ps2[0:16, :],
                                 w16("wssfe_h8", 128, 144, p=16),
                                 _win(xph, 16, 70 + fb),
                                 start=False, stop=True)
                nc.scalar.activation(_win(XSH, 16, 35 + fb), ps2[0:16, :],
                                     AF.Relu, bias=w32("bssfe_hi", p=16),
                                     scale=w32("gssfe_hi", p=16))
            for t in range(8):
                off = TAPS[t]
                nc.sync.dma_start(XSH9[16 * t:16 * t + 16, 0:PN - off],
                                  XSH[0:16, off:PN])
            # cc 1x1 144->128 -> t0
            t0 = tp.tile([128, N], F32R, tag="t", bufs=5, name="t0")
            for fc in range(2):
                fb = fc * 544
                ps = psV.tile([128, 512], F32, tag="vps", name="ps_cc")
                nc.tensor.matmul(ps[:], w16("wcc_lo"), _win(XSL, 128, 35 + fb),
                                 start=True, stop=False)
                nc.tensor.matmul(ps[:], w16("wcc_hi", p=16),
                                 _win(XSH, 16, 35 + fb),
                                 start=False, stop=True)
                nc.scalar.activation(t0[:, fc * 512:fc * 512 + 512], ps[:],
                                     AF.Relu, bias=w32("bcc"),
                                     scale=w32("gcc"))
            # cs conv 144->128 -> xp1
            for fc in range(2):
                fb = fc * 544
                ps = psV.tile([128, 512], F32, tag="vps", name="ps_cs")
                for tap in range(9):
                    nc.tensor.matmul(ps[:], w16("wcs_lo", tap * 128,
                                                tap * 128 + 128),
                                     _win(XSL, 128, TAPS[tap] + fb),
                                     start=(tap == 0), stop=False)
                nc.tensor.matmul(ps[:], w16("wcs_h9"), _win(XSH9, 128, fb),
                                 start=False, stop=False)
                nc.tensor.matmul(ps[:], w16("wcs_h8", p=16),
                                 _win(XSH, 16, 70 + fb),
                                 start=False, stop=True)
                nc.scalar.activation(_win(xp1, 128, 35 + fb), ps[:], AF.Relu,
                                     bias=w32("bcs"), scale=w32("gcs"))
            if i + 1 < BPC:
                load_img(i + 1)

            # ================= D1: LN1 -> cur =================
            t2 = scr.tile([128, N], F32R, tag="sd", bufs=2, name="t2")
            nc.gpsimd.tensor_mul(t2[:], t0[:].bitcast(F32), t0[:].bitcast(F32))
            cur = dw.tile([128, N], BF16, tag="cur", bufs=2, name="cur")
            for fc in range(2):
                sl = slice(fc * 512, fc * 512 + 512)
                mps, rstd = ln_pair(t0, t2, fc)
                tmm = scr.tile([128, 512], F32, tag="hd", bufs=3, name="tmm")
                nc.vector.tensor_sub(tmm[:], t0[:, sl].bitcast(F32), mps[:])
                tm2 = scr.tile([128, 512], F32, tag="hd", bufs=3, name="tm2")
                nc.vector.tensor_mul(tm2[:], tmm[:], rstd[:])
                nc.scalar.activation(cur[:, sl], tm2[:], AF.Identity,
                                     bias=w32("ln1b"), scale=w32("ln1g"))

            # ================= E1: lfe0 -> xp2 =================
            xp2 = dw.tile([128, N], BF16, tag="xp2", bufs=2, name="xp2")
            for fc in range(2):
                fb = fc * 544
                ps = psV.tile([128, 512], F32, tag="vps", name="ps_l0")
                for tap in range(9):
                    nc.tensor.matmul(ps[:], w16("wlfe0", tap * 128,
                                                tap * 128 + 128),
                                     _win(xp1, 128, TAPS[tap] + fb),
                                     start=(tap == 0), stop=(tap == 8))
                r0 = scr.tile([128, 512], F32, tag="he", bufs=3, name="r0")
                nc.scalar.activation(r0[:], ps[:], AF.Relu,
                                     bias=w32("blfe0"), scale=w32("glfe0"))
                nc.gpsimd.tensor_add(xp2[:, fc * 512:fc * 512 + 512], r0[:],
                                     _win(xp1, 128, 35 + fb))

            # ================= D2: gissa part 1 =================
            def gissa_qkA(src, wbd, bqk, mask, scale):
                Sps = psT.tile([128, 128], F32, tag="tps", name="Sps")
                for j in range(8):
                    qps = psD.tile([128, 256], F32, tag="dps", name="qps")
                    nc.tensor.matmul(qps[:], src[:, j * 128:(j + 1) * 128],
                                     wbd, start=True, stop=True)
                    qk = sm.tile([128, 256], BF16, tag="qk", bufs=3,
                                 name="qk")
                    nc.vector.tensor_add(qk[:], qps[:], bqk)
                    nc.tensor.matmul(Sps[:], qk[:, 0:128], qk[:, 128:256],
                                     start=(j == 0), stop=(j == 7))
                return ss_attn(Sps, mask, scale)

            def gissa_v(src, wv, bv):
                v = dw.tile([128, N], BF16, tag="gv", bufs=2, name="gv")
                for fc in range(2):
                    sl = slice(fc * 512, fc * 512 + 512)
                    ps = psD.tile([128, 512], F32, tag="dps", name="vps_")
                    nc.tensor.matmul(ps[:], wv, src[:, sl],
                                     start=True, stop=True)
                    nc.vector.tensor_scalar_add(v[:, sl], ps[:], bv)
                return v

            A1 = gissa_qkA(cur, w16("W1bd", 0, 256), BBR[:, 0:256],
                           w32("mask1"), S_HD)
            v1 = gissa_v(cur, w16("W1bd", 256, 384), w32("b1v"))

            # ================= E2: s0 q + vT =================
            q = dw.tile([128, N], BF16, tag="q", bufs=2, name="q")
            for fc in range(2):
                sl = slice(fc * 512, fc * 512 + 512)
                ps = psE.tile([128, 512], F32, tag="eps", name="ps_q")
                nc.tensor.matmul(ps[:], w16("Ws0piq"), xp2[:, sl],
                                 start=True, stop=True)
                nc.vector.tensor_scalar_add(q[:, sl], ps[:], w32("bs0piq"))
            vT = dw.tile([128, 8, 128], BF16, tag="vT", bufs=2, name="vT")
            for h in range(2):
                vp = psE.tile([128, 512], F32, tag="eps", name="vp")
                for jj in range(4):
                    j = h * 4 + jj
                    nc.tensor.matmul(vp[:, jj * 128:jj * 128 + 128],
                                     xp2[:, j * 128:(j + 1) * 128],
                                     w16("Ws0piv"), start=True, stop=True)
                    nc.vector.tensor_add(vT[:, j, :],
                                         vp[:, jj * 128:jj * 128 + 128],
                                         BBR[:, 512:640])

            # ================= D3: x1, shuffle, y/xr/t1 =================
            pT = psT.tile([128, 128], BF16, tag="tps", name="pT")
            nc.tensor.matmul(pT[:], A1[:], IDENT, is_transpose=True)
            AT = sm.tile([128, 128], BF16, tag="ssA", bufs=2, name="AT")
            nc.vector.tensor_copy(AT[:], pT[:])
            x1 = dw.tile([128, N], BF16, tag="x1", name="x1")
            for fc in range(2):
                sl = slice(fc * 512, fc * 512 + 512)
                ops = psD.tile([128, 512], F32, tag="dps", name="x1ps")
                nc.tensor.matmul(ops[:], AT[:], v1[:, sl],
                                 start=True, stop=True)
                nc.vector.scalar_tensor_tensor(x1[:, sl], ops[:], 1.0,
                                               cur[:, sl],
                                               op0=OP.mult, op1=OP.add)
            y = dw.tile([128, N], F32, tag="y", name="y")
            xr = dw.tile([128, N], BF16, tag="xr", name="xr")
            for fc in range(2):
                sl = slice(fc * 512, fc * 512 + 512)
                pps = psD.tile([128, 512], F32, tag="dps", name="shps")
                nc.tensor.matmul(pps[:], w16("P_a"), x1[:, sl],
                                 start=True, stop=True)
                nc.scalar.activation(y[:, sl], pps[:], AF.Identity,
                                     bias=w32("gbnb"), scale=w32("gbng"))
                nc.scalar.activation(xr[:, sl], pps[:], AF.Relu,
                                     bias=w32("gbnb"), scale=w32("gbng"))
            t1 = tp.tile([128, N], F32R, tag="t", bufs=5, name="t1")
            nc.gpsimd.tensor_add(t1[:], y[:], t0[:].bitcast(F32))

            # ================= D4: gissa part 2 =================
            A2 = gissa_qkA(xr, w16("W2bd", 0, 256), BBR[:, 256:512],
                           w32("mask2"), S_HEADS)
            v2 = gissa_v(xr, w16("W2bd", 256, 384), w32("b2v"))

            # ================= E3: s0 scores + exp =================
            Et = dw.tile([128, 8, N], BF16, tag="Et", name="Et")
            acc = sm.tile([128, 16], F32, tag="acc", bufs=2, name="acc")
            for j in range(8):
                for fc in range(2):
                    sl = slice(fc * 512, fc * 512 + 512)
                    sps = psE.tile([128, 512], F32, tag="eps", name="scps")
                    nc.tensor.matmul(sps[:], q[:, j * 128:(j + 1) * 128],
                                     q[:, sl], start=True, stop=True)
                    nc.scalar.activation(
                        Et[:, j, sl], sps[:], AF.Exp, scale=S_C,
                        accum_out=acc[:, fc * 8 + j:fc * 8 + j + 1])

            # ================= D5: o2 -> t2t =================
            Sp = psT.tile([128, 128], F32, tag="tps", name="Sp")
            nc.tensor.matmul(Sp[:], A2[:], w16("P_b"), start=True, stop=True)
            Ssb = sm.tile([128, 128], BF16, tag="ssA", bufs=2, name="Ssb")
            nc.vector.tensor_copy(Ssb[:], Sp[:])
            t2t = tp.tile([128, N], F32R, tag="t", bufs=5, name="t2t")
            for fc in range(2):
                sl = slice(fc * 512, fc * 512 + 512)
                ops = psD.tile([128, 512], F32, tag="dps", name="o2ps")
                nc.tensor.matmul(ops[:], Ssb[:], v2[:, sl],
                                 start=True, stop=True)
                nc.vector.scalar_tensor_tensor(t2t[:, sl], ops[:], 1.0,
                                               t1[:, sl].bitcast(F32),
                                               op0=OP.mult, op1=OP.add)

            # ================= E4: s0 den + O + po -> xp3 =================
            den8 = sm.tile([128, 8], F32, tag="den", bufs=4, name="den8")
            nc.vector.tensor_add(den8[:], acc[:, 0:8], acc[:, 8:16])
            denr = sm.tile([128, 8], F32, tag="den", bufs=4, name="denr")
            nc.vector.reciprocal(denr[:], den8[:])
            denrr = sm.tile([128, 8], BF16, tag="dnr", bufs=2, name="denrr")
            nc.vector.tensor_copy(denrr[:], denr[:])
            dT = psE.tile([8, 128], BF16, tag="eps", name="dT")
            nc.tensor.matmul(dT[:], denrr[:], IDENT, is_transpose=True)
            dt8 = sm.tile([8, 128], BF16, tag="dt8", bufs=2, name="dt8")
            nc.vector.tensor_copy(dt8[:], dT[:])
            rrep = dw.tile([128, N], F32, tag="rrep", bufs=2, name="rrep")
            for h in range(2):
                bc = psE.tile([128, 512], F32, tag="eps", name="bc")
                for jj in range(4):
                    j = h * 4 + jj
                    nc.tensor.matmul(bc[:, jj * 128:jj * 128 + 128],
                                     w16("sel8", j * 128, j * 128 + 128, p=8),
                                     dt8[0:8, :], start=True, stop=True)
                nc.scalar.activation(rrep[:, h * 512:h * 512 + 512], bc[:],
                                     AF.Copy)
            for fc in range(2):
                sl = slice(fc * 512, fc * 512 + 512)
                ops = psE.tile([128, 512], F32, tag="eps", name="Ops")
                for j in range(8):
                    nc.tensor.matmul(ops[:], vT[:, j, :], Et[:, j, sl],
                                     start=(j == 0), stop=(j == 7))
                Osb = scr.tile([128, 512], BF16, tag="hb", bufs=3, name="Osb")
                nc.scalar.activation(Osb[:], ops[:], AF.Copy)
                pps = psE.tile([128, 512], F32, tag="eps", name="pops")
                nc.tensor.matmul(pps[:], w16("Ws0po"), Osb[:],
                                 start=True, stop=True)
                tmp = scr.tile([128, 512], F32, tag="he", bufs=3, name="tmp")
                nc.vector.tensor_mul(tmp[:], pps[:], rrep[:, sl])
                nc.vector.scalar_tensor_tensor(
                    _win(xp3, 128, 35 + fc * 544), tmp[:], w32("bs0po"),
                    xp2[:, sl], op0=OP.add, op1=OP.add)

            # ================= D6: LN2 + MLP -> tf =================
            t2b = scr.tile([128, N], F32R, tag="sd", bufs=2, name="t2b")
            nc.gpsimd.tensor_mul(t2b[:], t2t[:].bitcast(F32),
                                 t2t[:].bitcast(F32))
            ln2 = dw.tile([128, N], BF16, tag="ln2", name="ln2")
            for fc in range(2):
                sl = slice(fc * 512, fc * 512 + 512)
                mps, rstd = ln_pair(t2t, t2b, fc)
                tmm = scr.tile([128, 512], F32, tag="hd", bufs=3, name="tmb")
                nc.vector.tensor_sub(tmm[:], t2t[:, sl].bitcast(F32), mps[:])
                nc.vector.tensor_mul(ln2[:, sl], tmm[:], rstd[:])

            # ================= E5: lfe1 -> xp4 =================
            xp4 = dw.tile([128, N], BF16, tag="xp4", name="xp4")
            for fc in range(2):
                fb = fc * 544
                ps = psV.tile([128, 512], F32, tag="vps", name="ps_l1")
                for tap in range(9):
                    nc.tensor.matmul(ps[:], w16("wlfe1", tap * 128,
                                                tap * 128 + 128),
                                     _win(xp3, 128, TAPS[tap] + fb),
                                     start=(tap == 0), stop=(tap == 8))
                r1 = scr.tile([128, 512], F32, tag="he", bufs=3, name="r1")
                nc.scalar.activation(r1[:], ps[:], AF.Relu,
                                     bias=w32("blfe1"), scale=w32("glfe1"))
                nc.gpsimd.tensor_add(xp4[:, fc * 512:fc * 512 + 512], r1[:],
                                     _win(xp3, 128, 35 + fb))

            # ================= E6: s1 -> xp5 =================
            vT2 = dw.tile([128, 8, 128], BF16, tag="vT", bufs=2, name="vT2")
            for h in range(2):
                vp = psE.tile([128, 512], F32, tag="eps", name="vp2")
                for jj in range(4):
                    j = h * 4 + jj
                    nc.tensor.matmul(vp[:, jj * 128:jj * 128 + 128],
                                     xp4[:, j * 128:(j + 1) * 128],
                                     w16("Ws1pi"), start=True, stop=True)
                    nc.vector.tensor_add(vT2[:, j, :],
                                         vp[:, jj * 128:jj * 128 + 128],
                                         BBR[:, 640:768])
            xp5 = dw.tile([128, N], F32, tag="xp5", name="xp5")
            for fc in range(2):
                sl = slice(fc * 512, fc * 512 + 512)
                ops = psE.tile([128, 512], F32, tag="eps", name="O2ps")
                for j in range(8):
                    nc.tensor.matmul(ops[:], vT2[:, j, :], Et[:, j, sl],
                                     start=(j == 0), stop=(j == 7))
                O2 = scr.tile([128, 512], BF16, tag="hb", bufs=3, name="O2")
                nc.scalar.activation(O2[:], ops[:], AF.Copy)
                pps = psE.tile([128, 512], F32, tag="eps", name="pops2")
                nc.tensor.matmul(pps[:], w16("Ws1po"), O2[:],
                                 start=True, stop=True)
                tmp = scr.tile([128, 512], F32, tag="he", bufs=3, name="tmp2")
                nc.vector.scalar_tensor_tensor(tmp[:], pps[:], S_C,
                                               rrep[:, sl], op0=OP.mult,
                                               op1=OP.mult)
                nc.vector.scalar_tensor_tensor(xp5[:, sl], tmp[:],
                                               w32("bs1po"),
                                               xp4[:, sl],
                                               op0=OP.add, op1=OP.add)

            # MLP last: tf only feeds the combine; gelus cluster while
            # the next image runs its (gelu-free) stem convs
            tf = tp.tile([128, N], F32R, tag="t", bufs=5, name="tf")
            for fc in range(2):
                sl = slice(fc * 512, fc * 512 + 512)
                wps = psT.tile([128, 512], F32, tag="tps", name="wps")
                for k in range(4):
                    gps = psD.tile([128, 512], F32, tag="dps", name="gps")
                    nc.tensor.matmul(gps[:], w16("W1m", k * 128, k * 128 + 128),
                                     ln2[:, sl], start=True, stop=True)
                    hk = scr.tile([128, 512], BF16, tag="hd", bufs=3,
                                  name="hk")
                    nc.scalar.activation(hk[:], gps[:], AF.Gelu,
                                         bias=w32("b1m")[:, k:k + 1])
                    nc.tensor.matmul(wps[:], w16("W2m", k * 128, k * 128 + 128),
                                     hk[:], start=(k == 0), stop=(k == 3))
                nc.vector.scalar_tensor_tensor(tf[:, sl], wps[:],
                                               w32("b2m"),
                                               t2t[:, sl].bitcast(F32),
                                               op0=OP.add, op1=OP.add)

            # ================= F: combine + pool =================
            u = scr.tile([128, N], F32, tag="sd", bufs=2, name="u")
            nc.vector.tensor_scalar_mul(u[:], xp5[:], w32("lamv")[:, 1:2])
            comb = scr.tile([128, N], F32, tag="sd", bufs=2, name="comb")
            nc.vector.scalar_tensor_tensor(comb[:], tf[:].bitcast(F32),
                                           w32("lamv")[:, 0:1], u[:],
                                           op0=OP.mult, op1=OP.add,
                                           accum_out=pooled[:, i:i + 1])

        # fc head: out[k, b] then DMA transposed
        fps = psT.tile([16, BPC], F32, tag="tps", name="fps")
        nc.tensor.matmul(fps[:], w32("fcw"), pooled[:], start=True, stop=True)
        fo = sm.tile([16, BPC], F32, tag="fo", name="fo")
        nc.vector.tensor_copy(fo[:], fps[:])
        oap = d_out.ap()  # [4, 16]
        nc.sync.dma_start(
            bass.AP(tensor=oap.tensor, offset=oap.offset,
                    ap=[[1, 16], [16, BPC]]),
            fo[:])

        for p in (psE, psT, psD, psV, sm, scr, dw, tp, wp):
            p.release()

    from concourse.hw_specs import get_activation_tables
    tabs = get_activation_tables(nc.m.arch)
    ln_exp = None
    for nm, s in tabs.items():
        if AF.Ln in s and AF.Exp in s:
            ln_exp = nm
            break
    if ln_exp is not None:
        for nm, s in tabs.items():
            if nm == ln_exp:
                continue
            s.discard(AF.Ln)
            s.discard(AF.Exp)
    nc.compile()
    return nc


def _prep(inputs):
    import ml_dtypes
    bf = ml_dtypes.bfloat16
    ii = {k: np.asarray(v, np.float32) for k, v in inputs.items()}
    lam = 1.0 / (1.0 + np.exp(-float(ii["lamuda"][0])))
    xpad = np.pad(ii["x"], ((0, 0), (0, 0), (1, 1), (1, 1)))
    xpf = xpad.reshape(32, 144, PN)
    xp9 = np.zeros((32, 128, PN), np.float32)
    for t in range(8):
        off = TAPS[t]
        xp9[:, 16 * t:16 * t + 16, 0:PN - off] = xpf[:, 128:144, off:PN]

    def c3(w):  # [o,i,3,3] -> [tap, i, o]
        return np.ascontiguousarray(
            w.transpose(2, 3, 1, 0).reshape(9, w.shape[1], w.shape[0]))

    wssfe = c3(ii["ssfe_w"]); wcs = c3(ii["cs_w"])
    wlfe0 = c3(ii["lfe0_w"]); wlfe1 = c3(ii["lfe1_w"])

    r = np.arange(128)
    W1bd = np.where((r[:, None] // 8) == (np.arange(384)[None, :] // 24),
                    ii["qkv_w"][np.arange(384)[None, :], r[:, None] % 8], 0.0)
    W2bd = np.where((r[:, None] // 16) == (np.arange(384)[None, :] // 48),
                    ii["qkv2_w"][np.arange(384)[None, :], r[:, None] % 16], 0.0)
    P_a = np.zeros((128, 128), np.float32)
    for h in range(16):
        for dd in range(8):
            P_a[h * 8 + dd, dd * 16 + h] = 1.0
    mask1 = ((r[:, None] // 8) == (r[None, :] // 8)).astype(np.float32)
    mask2 = ((r[:, None] // 16) == (r[None, :] // 16)).astype(np.float32)

    W1m = (ii["mlp_w1"] * ii["ln2_g"][None, :]).T            # [128c, 512]
    b1m = (ii["mlp_b1"] + ii["mlp_w1"] @ ii["ln2_b"]).reshape(4, 128).T
    W2m = ii["mlp_w2"].T.reshape(4, 128, 128).transpose(1, 0, 2)
    Ws0pi = (ii["s0_pi_w"] * ii["s0_bn_g"][:, None]).T       # [128c, 256o]
    bs0pi = ii["s0_pi_b"] * ii["s0_bn_g"] + ii["s0_bn_b"]
    Ws1pi = (ii["s1_pi_w"] * ii["s1_bn_g"][:, None]).T
    bs1pi = ii["s1_pi_b"] * ii["s1_bn_g"] + ii["s1_bn_b"]

    def blk16(nm):  # -> [rows, cols] f32
        if nm == "wssfe_lo":
            v = np.zeros((128, 1296), np.float32)
            for t in range(9):
                v[:, t * 144:t * 144 + 144] = wssfe[t, 0:128]
            return v
        if nm == "wssfe_h9":
            return wssfe[0:8, 128:144].reshape(128, 144)
        if nm == "wssfe_h8":
            return wssfe[8, 128:144]
        if nm == "wcc_lo":
            return ii["cc_w"].T[0:128]
        if nm == "wcc_hi":
            return ii["cc_w"].T[128:144]
        if nm == "wcs_lo":
            v = np.zeros((128, 1152), np.float32)
            for t in range(9):
                v[:, t * 128:t * 128 + 128] = wcs[t, 0:128]
            return v
        if nm == "wcs_h9":
            return wcs[0:8, 128:144].reshape(128, 128)
        if nm == "wcs_h8":
            return wcs[8, 128:144]
        if nm in ("wlfe0", "wlfe1"):
            w9 = wlfe0 if nm == "wlfe0" else wlfe1
            v = np.zeros((128, 1152), np.float32)
            for t in range(9):
                v[:, t * 128:t * 128 + 128] = w9[t]
            return v
        if nm == "W1bd":
            return W1bd
        if nm == "W2bd":
            return W2bd
        if nm == "W1m":
            return W1m
        if nm == "W2m":
            return W2m.reshape(128, 512)
        if nm == "Ws0piq":
            return Ws0pi[:, 0:128]
        if nm == "Ws0piv":
            return Ws0pi[:, 128:256]
        if nm == "Ws0po":
            return ii["s0_po_w"].T
        if nm == "Ws1pi":
            return Ws1pi
        if nm == "Ws1po":
            return ii["s1_po_w"].T
        if nm == "P_a":
            return P_a
        if nm == "P_b":
            return P_a.T
        if nm == "P_a":
            return P_a
        if nm == "W1bd":
            return W1bd
        if nm == "W2bd":
            return W2bd
        if nm == "W1m":
            return W1m
        if nm == "W2m":
            return W2m.reshape(128, 512)
        if nm == "sel8":
            v = np.zeros((8, 1024), np.float32)
            for j in range(8):
                v[j, j * 128:(j + 1) * 128] = 1.0
            return v
        if nm == "meanmat":
            return np.full((128, 128), 1.0 / 128.0, np.float32)
        if nm == "ones8r":
            return np.ones((8, 128), np.float32)
        if nm == "ident":
            return np.eye(128, dtype=np.float32)
        raise KeyError(nm)

    pk16 = np.zeros((128, _N16), np.float32)
    for nm, (c0, w) in _C16.items():
        v = blk16(nm)
        pk16[0:v.shape[0], c0:c0 + w] = v

    rblk = {
        "meanmat": np.full((128, 128), 1.0 / 128.0, np.float32),
    }
    pkr = np.zeros((128, _NR), np.float32)
    for nm, (c0, w) in _CR.items():
        v = rblk[nm]
        pkr[0:v.shape[0], c0:c0 + w] = v

    s32 = {
        "mask1": mask1, "mask2": mask2,
        "fcw": np.ascontiguousarray(ii["fc_w"].T) / 1024.0,
        "b1m": b1m,
        "lamv": np.stack([np.full(128, lam, np.float32),
                          np.full(128, 1 - lam, np.float32)], 1),
        "gssfe_lo": ii["ssfe_g"][0:128], "bssfe_lo": ii["ssfe_b"][0:128],
        "gssfe_hi": ii["ssfe_g"][128:144], "bssfe_hi": ii["ssfe_b"][128:144],
        "gcc": ii["cc_g"], "bcc": ii["cc_b"],
        "gcs": ii["cs_g"], "bcs": ii["cs_b"],
        "glfe0": ii["lfe0_g"], "blfe0": ii["lfe0_b"],
        "glfe1": ii["lfe1_g"], "blfe1": ii["lfe1_b"],
        "ln1g": ii["ln1_g"], "ln1b": ii["ln1_b"],
        "gbng": ii["gbn_g"], "gbnb": ii["gbn_b"],
        "b1v": ii["qkv_b"][256:384], "b2v": ii["qkv2_b"][256:384],
        "b2m": ii["mlp_b2"],
        "bs0piq": bs0pi[0:128], "bs0po": ii["s0_po_b"],
        "bs1po": ii["s1_po_b"],
        "epsb": np.full(128, EPS, np.float32),
    }
    pk32 = np.zeros((128, _N32), np.float32)
    for nm, (c0, w) in _C32.items():
        v = np.asarray(s32[nm], np.float32)
        if v.ndim == 1:
            v = v[:, None]
        pk32[0:v.shape[0], c0:c0 + w] = v

    bbr = np.concatenate([ii["qkv_b"][0:256], ii["qkv2_b"][0:256],
                          bs0pi[128:256], bs1pi]).reshape(1, 768)

    com = {
        "pk16": pk16.astype(bf),
        "pk32": pk32,
        "pkr": _rr(pkr),
        "bbr": np.ascontiguousarray(bbr, np.float32),
    }
    in_maps = []
    for c in range(NCORES):
        m = dict(com)
        m["xpad"] = xpf[c * BPC:(c + 1) * BPC].astype(bf)
        m["xpad9"] = xp9[c * BPC:(c + 1) * BPC].astype(bf)
        in_maps.append(m)
    return in_maps


def _run(inputs, trace=False):
    global _COMPILED
    if _COMPILED is None:
        _COMPILED = _build()
    in_maps = _prep(inputs)
    res = bass_utils.run_bass_kernel_spmd(
        _COMPILED, in_maps, core_ids=list(range(NCORES)), trace=trace)
    out = np.concatenate([r["out"] for r in res.results], 0)
    return out.astype(np.float32), res


def kernel(**inputs):
    out, _ = _run(inputs, trace=False)
    return out


# revision 28
# speedup vs baseline: 1.0375x; 1.0375x over previous
"""Trainium2 Bass kernel for nn_EATN (dense_transformer) — v3.

Data-parallel over batch: 32 images -> 8 NeuronCores x 4 images.
Layout: channel-major [C<=144 partitions, N=1024 free] throughout.

v3 vs v2 (scheduler-friendly restructure):
- per-branch PSUM pools (vps/dps/tps/eps) + per-branch SBUF tags so the
  tile list scheduler can overlap the channel branch (LN/GISSA/MLP
  chains) with the spatial branch (convs/SWSA) and the next image's
  stem convs; issue order interleaves the branches as a priority hint.
- all weights packed into one bf16 [128, 8624] tile (2 DMAs) + one f32
  [128, 301] tile; image-0 input DMA issued first so compute starts
  early instead of behind ~60 weight DMAs.
- bf16 stationaries (half LDWEIGHTS) and bf16 movers for every matmul
  with free < 256 (fp32r pays 4x cycles there).
- sqrt/rsqrt via Ln+Exp so the natural_log_exp act table stays
  resident; only Gelu switches tables (2 loads/image instead of ~7).
- softmax stabilization dropped: max exp argument ~25 so unnormalized
  exp stays far below f32/bf16 range; normalization cancels offsets.
- conv-input tiles (xpad/xsl/xsh/xsh9) in bf16 (residuals stay f32r).
- some elementwise adds/muls moved to the idle GPSIMD (Pool) engine.
"""

import sys

if "/opt/trn_rl_repo" not in sys.path:
    sys.path.insert(0, "/opt/trn_rl_repo")

import numpy as np

import concourse.bass as bass
import concourse.tile as tile
from concourse import bacc, mybir
from concourse import bass_utils

F32 = mybir.dt.float32
F32R = mybir.dt.float32r
BF16 = mybir.dt.bfloat16
U32 = mybir.dt.uint32
AF = mybir.ActivationFunctionType
OP = mybir.AluOpType

NCORES = 8
BPC = 4          # images per core
C = 128
N = 1024
PW = 34          # padded width
PN = PW * PW     # 1156
EPS = 1e-5
S_HD = 8.0 ** -0.5
S_HEADS = 16.0 ** -0.5
S_C = 128.0 ** -0.5
TAPS = [dy * PW + dx for dy in range(3) for dx in range(3)]  # 0..70

_COMPILED = None

# bf16 weight-pack column layout (chunk A = stem weights, chunk B = rest).
# Matmult operands must be both-32-bit or both-16-bit (NCC_IBIR034), so
# weights pairing with f32r streams live in the separate f32r pack below.
_L16 = [
    ("wssfe_lo", 1296), ("wssfe_h9", 144), ("wssfe_h8", 144),
    ("wcc_lo", 128), ("wcc_hi", 128),
    ("wcs_lo", 1152), ("wcs_h9", 128), ("wcs_h8", 128),     # A = 3248
    ("wlfe0", 1152), ("wlfe1", 1152),
    ("Ws0piq", 128), ("Ws0piv", 128), ("Ws0po", 128),
    ("Ws1pi", 128), ("Ws1po", 128),
    ("P_b", 128), ("ident", 128),
    ("W1bd", 384), ("W2bd", 384), ("W1m", 512), ("W2m", 512),
    ("P_a", 128), ("sel8", 1024),
]
_A16 = 3248
_N16 = sum(w for _, w in _L16)

_LR = [("meanmat", 128)]
_NR = sum(w for _, w in _LR)

_L32 = [("mask1", 128), ("mask2", 128), ("fcw", 16), ("b1m", 4),
        ("lamv", 2)] + [(nm, 1) for nm in (
            "gssfe_lo", "bssfe_lo", "gssfe_hi", "bssfe_hi",
            "gcc", "bcc", "gcs", "bcs", "glfe0", "blfe0", "glfe1", "blfe1",
            "ln1g", "ln1b", "gbng", "gbnb", "b1v", "b2v", "b2m",
            "bs0piq", "bs0po", "bs1po", "epsb")]
_N32 = sum(w for _, w in _L32)           # 301


def _cols(layout):
    off, d = 0, {}
    for nm, w in layout:
        d[nm] = (off, w)
        off += w
    return d


_C16 = _cols(_L16)
_C32 = _cols(_L32)
_CR = _cols(_LR)


def _rr(a):
    """Host-side RNE rounding to the fp32r grid (drop low 12 mantissa bits)."""
    a = np.ascontiguousarray(a, np.float32)
    b = a.view(np.uint32).astype(np.uint64)
    add = np.uint64((1 << 11) - 1) + ((b >> np.uint64(12)) & np.uint64(1))
    out = ((b + add) >> np.uint64(12) << np.uint64(12)).astype(np.uint32)
    return out.view(np.float32).copy()


def _win(t, p, off, rows=16, cols=32, rs=PW):
    """Strided [p, rows, cols] window into a flat [p, 1156] padded tile."""
    a = t[0:p, :]
    return bass.AP(tensor=a.tensor, offset=a.offset + off,
                   ap=[a.ap[0], [rs, rows], [1, cols]])


def _build():
    # measured Pool elementwise throughput is ~1.3 ns/elem (not the spec's
    # 0.83); calibrate so the list scheduler orders around gpsimd correctly
    from concourse import hw_specs
    hw_specs.TRN2Spec.CYCLE_T[mybir.EngineType.Pool] = 1.31
    nc = bacc.Bacc("TRN2", target_bir_lowering=False, debug=False,
                   num_devices=NCORES)

    d = {
        "xpad": nc.dram_tensor("xpad", [BPC, 144, PN], BF16,
                               kind="ExternalInput"),
        "xpad9": nc.dram_tensor("xpad9", [BPC, 128, PN], BF16,
                                kind="ExternalInput"),
        "pk16": nc.dram_tensor("pk16", [128, _N16], BF16,
                               kind="ExternalInput"),
        "pk32": nc.dram_tensor("pk32", [128, _N32], F32,
                               kind="ExternalInput"),
        "pkr": nc.dram_tensor("pkr", [128, _NR], F32R,
                              kind="ExternalInput"),
        "bbr": nc.dram_tensor("bbr", [1, 768], F32, kind="ExternalInput"),
    }
    d_out = nc.dram_tensor("out", [BPC, 16], F32, kind="ExternalOutput")

    with tile.TileContext(nc) as tc:
        wp = tc.alloc_tile_pool(name="wp", bufs=1)
        tp = tc.alloc_tile_pool(name="tp", bufs=1)
        dw = tc.alloc_tile_pool(name="dw", bufs=1)
        scr = tc.alloc_tile_pool(name="scr", bufs=1)
        sm = tc.alloc_tile_pool(name="sm", bufs=1)
        psV = tc.alloc_tile_pool(name="psV", bufs=2, space="PSUM")
        psD = tc.alloc_tile_pool(name="psD", bufs=2, space="PSUM")
        psT = tc.alloc_tile_pool(name="psT", bufs=1, space="PSUM")
        psE = tc.alloc_tile_pool(name="psE", bufs=3, space="PSUM")

        # ---- persistent tiles; image-0 input DMA first, then weights
        XPL = [wp.tile([128, PN], BF16, tag=f"xpl{k}", name=f"xpl{k}")
               for k in range(2)]
        XPH = [wp.tile([16, PN], BF16, tag=f"xph{k}", name=f"xph{k}")
               for k in range(2)]
        XPH9 = [wp.tile([128, PN], BF16, tag=f"xph9{k}", name=f"xph9{k}")
                for k in range(2)]
        PK16 = wp.tile([128, _N16], BF16, tag="pk16", name="PK16")
        PK32 = wp.tile([128, _N32], F32, tag="pk32", name="PK32")
        PKR = wp.tile([128, _NR], F32R, tag="pkr", name="PKR")
        BBR = wp.tile([128, 768], F32, tag="bbr", name="BBR")

        def load_img(i):
            nc.sync.dma_start(XPL[i % 2][:], d["xpad"].ap()[i, 0:128])
            nc.sync.dma_start(XPH[i % 2][:], d["xpad"].ap()[i, 128:144])
            nc.sync.dma_start(XPH9[i % 2][:], d["xpad9"].ap()[i])

        load_img(0)
        nc.sync.dma_start(PK16[:, 0:_A16], d["pk16"].ap()[:, 0:_A16])
        nc.sync.dma_start(PK32[:], d["pk32"].ap())
        nc.sync.dma_start(BBR[:], d["bbr"].ap().to_broadcast([128, 768]))
        nc.sync.dma_start(PK16[:, _A16:_N16], d["pk16"].ap()[:, _A16:_N16])
        nc.sync.dma_start(PKR[:], d["pkr"].ap())

        def w16(nm, a=None, b=None, p=128):
            c0, w = _C16[nm]
            if a is None:
                a, b = 0, w
            return PK16[0:p, c0 + a:c0 + b]

        def w32(nm, p=128):
            c0, w = _C32[nm]
            return PK32[0:p, c0:c0 + w]

        def wr(nm, a=None, b=None, p=128):
            c0, w = _CR[nm]
            if a is None:
                a, b = 0, w
            return PKR[0:p, c0 + a:c0 + b]

        MEAN = wr("meanmat")
        IDENT = w16("ident")
        EPSB = w32("epsb")

        XSL = wp.tile([128, PN], BF16, tag="xsl", name="XSL")
        XSH = wp.tile([16, PN], BF16, tag="xsh", name="XSH")
        XSH9 = wp.tile([128, PN], BF16, tag="xsh9", name="XSH9")
        XP = [wp.tile([128, PN], BF16, tag=f"xp{k}", name=f"xp{k}")
              for k in range(3)]
        for t in (XSL, XSH, XSH9):
            nc.vector.memset(t[:], 0.0)
        for t in XP:
            nc.vector.memset(t[:], 0.0)
        pooled = wp.tile([128, BPC], F32, tag="pooled", name="pooled")

        def ln_pair(src, sq, fc):
            """mean/meansq matmuls + rstd for 512-chunk fc. Returns
            (mps[128,512] psum, rstd[128,512] sbuf)."""
            sl = slice(fc * 512, fc * 512 + 512)
            mps = psT.tile([128, 512], F32, tag="tps", name="mps")
            nc.tensor.matmul(mps[:], MEAN, src[:, sl], start=True, stop=True)
            sps = psD.tile([128, 512], F32, tag="dps", name="sps")
            nc.tensor.matmul(sps[:], MEAN, sq[:, sl], start=True, stop=True)
            b = scr.tile([128, 512], F32, tag="hd", bufs=3, name="b")
            nc.scalar.activation(b[:], mps[:], AF.Square)
            v = scr.tile([128, 512], F32, tag="hd", bufs=3, name="v")
            nc.vector.tensor_sub(v[:], sps[:], b[:])
            l = scr.tile([128, 512], F32, tag="hd", bufs=3, name="l")
            nc.scalar.activation(l[:], v[:], AF.Ln, bias=EPSB)
            rstd = scr.tile([128, 512], F32, tag="hd", bufs=3, name="rstd")
            nc.scalar.activation(rstd[:], l[:], AF.Exp, scale=-0.5)
            return mps, rstd

        def ss_attn(S_ps, mask, scale):
            """softmax(sign*sqrt(|scale*S|+eps)); A [128,128] bf16 sbuf."""
            r1 = sm.tile([128, 128], F32, tag="ssa", bufs=4, name="r1")
            nc.scalar.activation(r1[:], S_ps[:], AF.Abs, scale=scale)
            l = sm.tile([128, 128], F32, tag="ssa", bufs=4, name="l")
            nc.scalar.activation(l[:], r1[:], AF.Ln, bias=EPSB)
            r2 = sm.tile([128, 128], F32, tag="ssa", bufs=4, name="r2")
            nc.scalar.activation(r2[:], l[:], AF.Exp, scale=0.5)
            sb = sm.tile([128, 128], F32, tag="ssa", bufs=4, name="sb")
            nc.vector.tensor_single_scalar(sb[:].bitcast(U32),
                                           S_ps[:].bitcast(U32),
                                           0x80000000, op=OP.bitwise_and)
            g = sm.tile([128, 128], F32, tag="ssa", bufs=4, name="g")
            nc.vector.tensor_tensor(g[:].bitcast(U32), r2[:].bitcast(U32),
                                    sb[:].bitcast(U32), op=OP.bitwise_or)
            e = sm.tile([128, 128], F32, tag="ssa", bufs=4, name="e")
            nc.scalar.activation(e[:], g[:], AF.Exp)
            em = sm.tile([128, 128], F32, tag="ssa", bufs=4, name="em")
            rs = sm.tile([128, 1], F32, tag="sss", bufs=4, name="rs")
            nc.vector.scalar_tensor_tensor(em[:], e[:], 1.0, mask,
                                           op0=OP.mult, op1=OP.mult,
                                           accum_out=rs[:, 0:1])
            rr = sm.tile([128, 1], F32, tag="sss", bufs=4, name="rr")
            nc.vector.reciprocal(rr[:], rs[:])
            A = sm.tile([128, 128], BF16, tag="ssA", bufs=2, name="A")
            nc.vector.tensor_scalar_mul(A[:], em[:], rr[:, 0:1])
            return A

        for i in range(BPC):
            xpl, xph, xph9 = XPL[i % 2], XPH[i % 2], XPH9[i % 2]
            xp1, xp3 = XP[(2 * i) % 3], XP[(2 * i + 1) % 3]

            # ================= C: stem (ssfe, cc, cs) =================
            for fc in range(2):
                fb = fc * 544
                ps = psV.tile([128, 512], F32, tag="vps", name="ps_sl")
                for tap in range(9):
                    nc.tensor.matmul(ps[:], w16("wssfe_lo", tap * 144,
                                                tap * 144 + 128),
                                     _win(xpl, 128, TAPS[tap] + fb),
                                     start=(tap == 0), stop=False)
                nc.tensor.matmul(ps[:], w16("wssfe_h9", 0, 128),
                                 _win(xph9, 128, fb), start=False, stop=False)
                nc.tensor.matmul(ps[:], w16("wssfe_h8", 0, 128, p=16),
                                 _win(xph, 16, 70 + fb),
                                 start=False, stop=True)
                nc.scalar.activation(_win(XSL, 128, 35 + fb), ps[:], AF.Relu,
                                     bias=w32("bssfe_lo"),
                                     scale=w32("gssfe_lo"))
            for fc in range(2):
                fb = fc * 544
                ps2 = psV.tile([128, 512], F32, tag="vps", name="ps_sh")
                for tap in range(9):
                    nc.tensor.matmul(ps2[0:16, :],
                                     w16("wssfe_lo", tap * 144 + 128,
                                         tap * 144 + 144),
                                     _win(xpl, 128, TAPS[tap] + fb),
                                     start=(tap == 0), stop=False)
                nc.tensor.matmul(ps2[0:16, :], w16("wssfe_h9", 128, 144),
                                 _win(xph9, 128, fb), start=False, stop=False)
                nc.tensor.matmul(ps2[0:16, :],
                                 w16("wssfe_h8", 128, 144, p=16),
                                 _win(xph, 16, 70 + fb),
                                 start=False, stop=True)
                nc.scalar.activation(_win(XSH, 16, 35 + fb), ps2[0:16, :],
                                     AF.Relu, bias=w32("bssfe_hi", p=16),
                                     scale=w32("gssfe_hi", p=16))
            for t in range(8):
                off = TAPS[t]
                nc.sync.dma_start(XSH9[16 * t:16 * t + 16, 0:PN - off],
                                  XSH[0:16, off:PN])
            # cc 1x1 144->128 -> t0
            t0 = tp.tile([128, N], F32R, tag="t", bufs=5, name="t0")
            for fc in range(2):
                fb = fc * 544
                ps = psV.tile([128, 512], F32, tag="vps", name="ps_cc")
                nc.tensor.matmul(ps[:], w16("wcc_lo"), _win(XSL, 128, 35 + fb),
                                 start=True, stop=False)
                nc.tensor.matmul(ps[:], w16("wcc_hi", p=16),
                                 _win(XSH, 16, 35 + fb),
                                 start=False, stop=True)
                nc.scalar.activation(t0[:, fc * 512:fc * 512 + 512], ps[:],
                                     AF.Relu, bias=w32("bcc"),
                                     scale=w32("gcc"))
            # cs conv 144->128 -> xp1
            for fc in range(2):
                fb = fc * 544
                ps = psV.tile([128, 512], F32, tag="vps", name="ps_cs")
                for tap in range(9):
                    nc.tensor.matmul(ps[:], w16("wcs_lo", tap * 128,
                                                tap * 128 + 128),
                                     _win(XSL, 128, TAPS[tap] + fb),
                                     start=(tap == 0), stop=False)
                nc.tensor.matmul(ps[:], w16("wcs_h9"), _win(XSH9, 128, fb),
                                 start=False, stop=False)
                nc.tensor.matmul(ps[:], w16("wcs_h8", p=16),
                                 _win(XSH, 16, 70 + fb),
                                 start=False, stop=True)
                nc.scalar.activation(_win(xp1, 128, 35 + fb), ps[:], AF.Relu,
                                     bias=w32("bcs"), scale=w32("gcs"))
            if i + 1 < BPC:
                load_img(i + 1)

            # ================= D1: LN1 -> cur =================
            t2 = scr.tile([128, N], F32R, tag="sd", bufs=2, name="t2")
            nc.gpsimd.tensor_mul(t2[:], t0[:].bitcast(F32), t0[:].bitcast(F32))
            cur = dw.tile([128, N], BF16, tag="cur", bufs=2, name="cur")
            for fc in range(2):
                sl = slice(fc * 512, fc * 512 + 512)
                mps, rstd = ln_pair(t0, t2, fc)
                tmm = scr.tile([128, 512], F32, tag="hd", bufs=3, name="tmm")
                nc.vector.tensor_sub(tmm[:], t0[:, sl].bitcast(F32), mps[:])
                tm2 = scr.tile([128, 512], F32, tag="hd", bufs=3, name="tm2")
                nc.vector.tensor_mul(tm2[:], tmm[:], rstd[:])
                nc.vector.tensor_scalar(cur[:, sl], tm2[:], w32("ln1g"),
                                        w32("ln1b"), op0=OP.mult, op1=OP.add)

            # ================= E1: lfe0 -> xp2 =================
            xp2 = dw.tile([128, N], BF16, tag="xp2", bufs=2, name="xp2")
            for fc in range(2):
                fb = fc * 544
                ps = psV.tile([128, 512], F32, tag="vps", name="ps_l0")
                for tap in range(9):
                    nc.tensor.matmul(ps[:], w16("wlfe0", tap * 128,
                                                tap * 128 + 128),
                                     _win(xp1, 128, TAPS[tap] + fb),
                                     start=(tap == 0), stop=(tap == 8))
                r0 = scr.tile([128, 512], F32, tag="he", bufs=3, name="r0")
                nc.scalar.activation(r0[:], ps[:], AF.Relu,
                                     bias=w32("blfe0"), scale=w32("glfe0"))
                nc.gpsimd.tensor_add(xp2[:, fc * 512:fc * 512 + 512], r0[:],
                                     _win(xp1, 128, 35 + fb))

            # ================= D2: gissa part 1 =================
            def gissa_qkA(src, wbd, bqk, mask, scale):
                Sps = psT.tile([128, 128], F32, tag="tps", name="Sps")
                for j in range(8):
                    qps = psD.tile([128, 256], F32, tag="dps", name="qps")
                    nc.tensor.matmul(qps[:], src[:, j * 128:(j + 1) * 128],
                                     wbd, start=True, stop=True)
                    qk = sm.tile([128, 256], BF16, tag="qk", bufs=3,
                                 name="qk")
                    nc.vector.tensor_add(qk[:], qps[:], bqk)
                    nc.tensor.matmul(Sps[:], qk[:, 0:128], qk[:, 128:256],
                                     start=(j == 0), stop=(j == 7))
                return ss_attn(Sps, mask, scale)

            def gissa_v(src, wv, bv):
                v = dw.tile([128, N], BF16, tag="gv", bufs=2, name="gv")
                for fc in range(2):
                    sl = slice(fc * 512, fc * 512 + 512)
                    ps = psD.tile([128, 512], F32, tag="dps", name="vps_")
                    nc.tensor.matmul(ps[:], wv, src[:, sl],
                                     start=True, stop=True)
                    nc.vector.tensor_scalar_add(v[:, sl], ps[:], bv)
                return v

            A1 = gissa_qkA(cur, w16("W1bd", 0, 256), BBR[:, 0:256],
                           w32("mask1"), S_HD)
            v1 = gissa_v(cur, w16("W1bd", 256, 384), w32("b1v"))

            # ================= E2: s0 q + vT =================
            q = dw.tile([128, N], BF16, tag="q", bufs=2, name="q")
            for fc in range(2):
                sl = slice(fc * 512, fc * 512 + 512)
                ps = psE.tile([128, 512], F32, tag="eps", name="ps_q")
                nc.tensor.matmul(ps[:], w16("Ws0piq"), xp2[:, sl],
                                 start=True, stop=True)
                nc.vector.tensor_scalar_add(q[:, sl], ps[:], w32("bs0piq"))
            vT = dw.tile([128, 8, 128], BF16, tag="vT", bufs=2, name="vT")
            for h in range(2):
                vp = psE.tile([128, 512], F32, tag="eps", name="vp")
                for jj in range(4):
                    j = h * 4 + jj
                    nc.tensor.matmul(vp[:, jj * 128:jj * 128 + 128],
                                     xp2[:, j * 128:(j + 1) * 128],
                                     w16("Ws0piv"), start=True, stop=True)
                    nc.vector.tensor_add(vT[:, j, :],
                                         vp[:, jj * 128:jj * 128 + 128],
                                         BBR[:, 512:640])

            # ================= D3: x1, shuffle, y/xr/t1 =================
            pT = psT.tile([128, 128], BF16, tag="tps", name="pT")
            nc.tensor.matmul(pT[:], A1[:], IDENT, is_transpose=True)
            AT = sm.tile([128, 128], BF16, tag="ssA", bufs=2, name="AT")
            nc.vector.tensor_copy(AT[:], pT[:])
            x1 = dw.tile([128, N], BF16, tag="x1", name="x1")
            for fc in range(2):
                sl = slice(fc * 512, fc * 512 + 512)
                ops = psD.tile([128, 512], F32, tag="dps", name="x1ps")
                nc.tensor.matmul(ops[:], AT[:], v1[:, sl],
                                 start=True, stop=True)
                nc.vector.scalar_tensor_tensor(x1[:, sl], ops[:], 1.0,
                                               cur[:, sl],
                                               op0=OP.mult, op1=OP.add)
            y = dw.tile([128, N], F32, tag="y", name="y")
            xr = dw.tile([128, N], BF16, tag="xr", name="xr")
            for fc in range(2):
                sl = slice(fc * 512, fc * 512 + 512)
                pps = psD.tile([128, 512], F32, tag="dps", name="shps")
                nc.tensor.matmul(pps[:], w16("P_a"), x1[:, sl],
                                 start=True, stop=True)
                nc.vector.tensor_scalar(y[:, sl], pps[:], w32("gbng"),
                                        w32("gbnb"), op0=OP.mult, op1=OP.add)
                nc.scalar.activation(xr[:, sl], y[:, sl], AF.Relu)
            t1 = tp.tile([128, N], F32R, tag="t", bufs=5, name="t1")
            nc.gpsimd.tensor_add(t1[:], y[:], t0[:].bitcast(F32))

            # ================= D4: gissa part 2 =================
            A2 = gissa_qkA(xr, w16("W2bd", 0, 256), BBR[:, 256:512],
                           w32("mask2"), S_HEADS)
            v2 = gissa_v(xr, w16("W2bd", 256, 384), w32("b2v"))

            # ================= E3: s0 scores + exp =================
            Et = dw.tile([128, 8, N], BF16, tag="Et", name="Et")
            acc = sm.tile([128, 16], F32, tag="acc", bufs=2, name="acc")
            for j in range(8):
                for fc in range(2):
                    sl = slice(fc * 512, fc * 512 + 512)
                    sps = psE.tile([128, 512], F32, tag="eps", name="scps")
                    nc.tensor.matmul(sps[:], q[:, j * 128:(j + 1) * 128],
                                     q[:, sl], start=True, stop=True)
                    nc.scalar.activation(
                        Et[:, j, sl], sps[:], AF.Exp, scale=S_C,
                        accum_out=acc[:, fc * 8 + j:fc * 8 + j + 1])

            # ================= D5: o2 -> t2t =================
            Sp = psT.tile([128, 128], F32, tag="tps", name="Sp")
            nc.tensor.matmul(Sp[:], A2[:], w16("P_b"), start=True, stop=True)
            Ssb = sm.tile([128, 128], BF16, tag="ssA", bufs=2, name="Ssb")
            nc.vector.tensor_copy(Ssb[:], Sp[:])
            t2t = tp.tile([128, N], F32R, tag="t", bufs=5, name="t2t")
            for fc in range(2):
                sl = slice(fc * 512, fc * 512 + 512)
                ops = psD.tile([128, 512], F32, tag="dps", name="o2ps")
                nc.tensor.matmul(ops[:], Ssb[:], v2[:, sl],
                                 start=True, stop=True)
                nc.vector.scalar_tensor_tensor(t2t[:, sl], ops[:], 1.0,
                                               t1[:, sl].bitcast(F32),
                                               op0=OP.mult, op1=OP.add)

            # ================= E4: s0 den + O + po -> xp3 =================
            den8 = sm.tile([128, 8], F32, tag="den", bufs=4, name="den8")
            nc.vector.tensor_add(den8[:], acc[:, 0:8], acc[:, 8:16])
            denr = sm.tile([128, 8], F32, tag="den", bufs=4, name="denr")
            nc.vector.reciprocal(denr[:], den8[:])
            denrr = sm.tile([128, 8], BF16, tag="dnr", bufs=2, name="denrr")
            nc.vector.tensor_copy(denrr[:], denr[:])
            dT = psE.tile([8, 128], BF16, tag="eps", name="dT")
            nc.tensor.matmul(dT[:], denrr[:], IDENT, is_transpose=True)
            dt8 = sm.tile([8, 128], BF16, tag="dt8", bufs=2, name="dt8")
            nc.vector.tensor_copy(dt8[:], dT[:])
            rrep = dw.tile([128, N], F32, tag="rrep", bufs=2, name="rrep")
            for h in range(2):
                bc = psE.tile([128, 512], F32, tag="eps", name="bc")
                for jj in range(4):
                    j = h * 4 + jj
                    nc.tensor.matmul(bc[:, jj * 128:jj * 128 + 128],
                                     w16("sel8", j * 128, j * 128 + 128, p=8),
                                     dt8[0:8, :], start=True, stop=True)
                nc.scalar.activation(rrep[:, h * 512:h * 512 + 512], bc[:],
                                     AF.Copy)
            for fc in range(2):
                sl = slice(fc * 512, fc * 512 + 512)
                ops = psE.tile([128, 512], F32, tag="eps", name="Ops")
                for j in range(8):
                    nc.tensor.matmul(ops[:], vT[:, j, :], Et[:, j, sl],
                                     start=(j == 0), stop=(j == 7))
                Osb = scr.tile([128, 512], BF16, tag="hb", bufs=3, name="Osb")
                nc.scalar.activation(Osb[:], ops[:], AF.Copy)
                pps = psE.tile([128, 512], F32, tag="eps", name="pops")
                nc.tensor.matmul(pps[:], w16("Ws0po"), Osb[:],
                                 start=True, stop=True)
                tmp = scr.tile([128, 512], F32, tag="he", bufs=3, name="tmp")
                nc.vector.tensor_mul(tmp[:], pps[:], rrep[:, sl])
                nc.vector.scalar_tensor_tensor(
                    _win(xp3, 128, 35 + fc * 544), tmp[:], w32("bs0po"),
                    xp2[:, sl], op0=OP.add, op1=OP.add)

            # ================= D6: LN2 + MLP -> tf =================
            t2b = scr.tile([128, N], F32R, tag="sd", bufs=2, name="t2b")
            nc.gpsimd.tensor_mul(t2b[:], t2t[:].bitcast(F32),
                                 t2t[:].bitcast(F32))
            ln2 = dw.tile([128, N], BF16, tag="ln2", name="ln2")
            for fc in range(2):
                sl = slice(fc * 512, fc * 512 + 512)
                mps, rstd = ln_pair(t2t, t2b, fc)
                tmm = scr.tile([128, 512], F32, tag="hd", bufs=3, name="tmb")
                nc.vector.tensor_sub(tmm[:], t2t[:, sl].bitcast(F32), mps[:])
                nc.vector.tensor_mul(ln2[:, sl], tmm[:], rstd[:])

            # ================= E5: lfe1 -> xp4 =================
            xp4 = dw.tile([128, N], BF16, tag="xp4", name="xp4")
            for fc in range(2):
                fb = fc * 544
                ps = psV.tile([128, 512], F32, tag="vps", name="ps_l1")
                for tap in range(9):
                    nc.tensor.matmul(ps[:], w16("wlfe1", tap * 128,
                                                tap * 128 + 128),
                                     _win(xp3, 128, TAPS[tap] + fb),
                                     start=(tap == 0), stop=(tap == 8))
                r1 = scr.tile([128, 512], F32, tag="he", bufs=3, name="r1")
                nc.scalar.activation(r1[:], ps[:], AF.Relu,
                                     bias=w32("blfe1"), scale=w32("glfe1"))
                nc.gpsimd.tensor_add(xp4[:, fc * 512:fc * 512 + 512], r1[:],
                                     _win(xp3, 128, 35 + fb))

            # ================= E6: s1 -> xp5 =================
            vT2 = dw.tile([128, 8, 128], BF16, tag="vT", bufs=2, name="vT2")
            for h in range(2):
                vp = psE.tile([128, 512], F32, tag="eps", name="vp2")
                for jj in range(4):
                    j = h * 4 + jj
                    nc.tensor.matmul(vp[:, jj * 128:jj * 128 + 128],
                                     xp4[:, j * 128:(j + 1) * 128],
                                     w16("Ws1pi"), start=True, stop=True)
                    nc.vector.tensor_add(vT2[:, j, :],
                                         vp[:, jj * 128:jj * 128 + 128],
                                         BBR[:, 640:768])
            xp5 = dw.tile([128, N], F32, tag="xp5", name="xp5")
            for fc in range(2):
                sl = slice(fc * 512, fc * 512 + 512)
                ops = psE.tile([128, 512], F32, tag="eps", name="O2ps")
                for j in range(8):
                    nc.tensor.matmul(ops[:], vT2[:, j, :], Et[:, j, sl],
                                     start=(j == 0), stop=(j == 7))
                O2 = scr.tile([128, 512], BF16, tag="hb", bufs=3, name="O2")
                nc.scalar.activation(O2[:], ops[:], AF.Copy)
                pps = psE.tile([128, 512], F32, tag="eps", name="pops2")
                nc.tensor.matmul(pps[:], w16("Ws1po"), O2[:],
                                 start=True, stop=True)
                tmp = scr.tile([128, 512], F32, tag="he", bufs=3, name="tmp2")
                nc.vector.scalar_tensor_tensor(tmp[:], pps[:], S_C,
                                               rrep[:, sl], op0=OP.mult,
                                               op1=OP.mult)
                nc.vector.scalar_tensor_tensor(xp5[:, sl], tmp[:],
                                               w32("bs1po"),
                                               xp4[:, sl],
                                               op0=OP.add, op1=OP.add)

            # MLP last: tf only feeds the combine; gelus cluster while
            # the next image runs its (gelu-free) stem convs
            tf = tp.tile([128, N], F32R, tag="t", bufs=5, name="tf")
            for fc in range(2):
                sl = slice(fc * 512, fc * 512 + 512)
                wps = psT.tile([128, 512], F32, tag="tps", name="wps")
                for k in range(4):
                    gps = psD.tile([128, 512], F32, tag="dps", name="gps")
                    nc.tensor.matmul(gps[:], w16("W1m", k * 128, k * 128 + 128),
                                     ln2[:, sl], start=True, stop=True)
                    hk = scr.tile([128, 512], BF16, tag="hd", bufs=3,
                                  name="hk")
                    nc.scalar.activation(hk[:], gps[:], AF.Gelu,
                                         bias=w32("b1m")[:, k:k + 1])
                    nc.tensor.matmul(wps[:], w16("W2m", k * 128, k * 128 + 128),
                                     hk[:], start=(k == 0), stop=(k == 3))
                nc.vector.scalar_tensor_tensor(tf[:, sl], wps[:],
                                               w32("b2m"),
                                               t2t[:, sl].bitcast(F32),
                                               op0=OP.add, op1=OP.add)

            # ================= F: combine + pool =================
            u = scr.tile([128, N], F32, tag="sd", bufs=2, name="u")
            nc.vector.tensor_scalar_mul(u[:], xp5[:], w32("lamv")[:, 1:2])
            comb = scr.tile([128, N], F32, tag="sd", bufs=2, name="comb")
            nc.vector.scalar_tensor_tensor(comb[:], tf[:].bitcast(F32),
                                           w32("lamv")[:, 0:1], u[:],
                                           op0=OP.mult, op1=OP.add,
                                           accum_out=pooled[:, i:i + 1])

        # fc head: out[k, b] then DMA transposed
        fps = psT.tile([16, BPC], F32, tag="tps", name="fps")
        nc.tensor.matmul(fps[:], w32("fcw"), pooled[:], start=True, stop=True)
        fo = sm.tile([16, BPC], F32, tag="fo", name="fo")
        nc.vector.tensor_copy(fo[:], fps[:])
        oap = d_out.ap()  # [4, 16]
        nc.sync.dma_start(
            bass.AP(tensor=oap.tensor, offset=oap.offset,
                    ap=[[1, 16], [16, BPC]]),
            fo[:])

        for p in (psE, psT, psD, psV, sm, scr, dw, tp, wp):
            p.release()

    from concourse.hw_specs import get_activation_tables
    tabs = get_activation_tables(nc.m.arch)
    ln_exp = None
    for nm, s in tabs.items():
        if AF.Ln in s and AF.Exp in s:
            ln_exp = nm
            break
    if ln_exp is not None:
        for nm, s in tabs.items():
            if nm == ln_exp:
                continue
            s.discard(AF.Ln)
            s.discard(AF.Exp)
    nc.compile()
    return nc


def _prep(inputs):
    import ml_dtypes
    bf = ml_dtypes.bfloat16
    ii = {k: np.asarray(v, np.float32) for k, v in inputs.items()}
    lam = 1.0 / (1.0 + np.exp(-float(ii["lamuda"][0])))
    xpad = np.pad(ii["x"], ((0, 0), (0, 0), (1, 1), (1, 1)))
    xpf = xpad.reshape(32, 144, PN)
    xp9 = np.zeros((32, 128, PN), np.float32)
    for t in range(8):
        off = TAPS[t]
        xp9[:, 16 * t:16 * t + 16, 0:PN - off] = xpf[:, 128:144, off:PN]

    def c3(w):  # [o,i,3,3] -> [tap, i, o]
        return np.ascontiguousarray(
            w.transpose(2, 3, 1, 0).reshape(9, w.shape[1], w.shape[0]))

    wssfe = c3(ii["ssfe_w"]); wcs = c3(ii["cs_w"])
    wlfe0 = c3(ii["lfe0_w"]); wlfe1 = c3(ii["lfe1_w"])

    r = np.arange(128)
    W1bd = np.where((r[:, None] // 8) == (np.arange(384)[None, :] // 24),
                    ii["qkv_w"][np.arange(384)[None, :], r[:, None] % 8], 0.0)
    W2bd = np.where((r[:, None] // 16) == (np.arange(384)[None, :] // 48),
                    ii["qkv2_w"][np.arange(384)[None, :], r[:, None] % 16], 0.0)
    P_a = np.zeros((128, 128), np.float32)
    for h in range(16):
        for dd in range(8):
            P_a[h * 8 + dd, dd * 16 + h] = 1.0
    mask1 = ((r[:, None] // 8) == (r[None, :] // 8)).astype(np.float32)
    mask2 = ((r[:, None] // 16) == (r[None, :] // 16)).astype(np.float32)

    W1m = (ii["mlp_w1"] * ii["ln2_g"][None, :]).T            # [128c, 512]
    b1m = (ii["mlp_b1"] + ii["mlp_w1"] @ ii["ln2_b"]).reshape(4, 128).T
    W2m = ii["mlp_w2"].T.reshape(4, 128, 128).transpose(1, 0, 2)
    Ws0pi = (ii["s0_pi_w"] * ii["s0_bn_g"][:, None]).T       # [128c, 256o]
    bs0pi = ii["s0_pi_b"] * ii["s0_bn_g"] + ii["s0_bn_b"]
    Ws1pi = (ii["s1_pi_w"] * ii["s1_bn_g"][:, None]).T
    bs1pi = ii["s1_pi_b"] * ii["s1_bn_g"] + ii["s1_bn_b"]

    def blk16(nm):  # -> [rows, cols] f32
        if nm == "wssfe_lo":
            v = np.zeros((128, 1296), np.float32)
            for t in range(9):
                v[:, t * 144:t * 144 + 144] = wssfe[t, 0:128]
            return v
        if nm == "wssfe_h9":
            return wssfe[0:8, 128:144].reshape(128, 144)
        if nm == "wssfe_h8":
            return wssfe[8, 128:144]
        if nm == "wcc_lo":
            return ii["cc_w"].T[0:128]
        if nm == "wcc_hi":
            return ii["cc_w"].T[128:144]
        if nm == "wcs_lo":
            v = np.zeros((128, 1152), np.float32)
            for t in range(9):
                v[:, t * 128:t * 128 + 128] = wcs[t, 0:128]
            return v
        if nm == "wcs_h9":
            return wcs[0:8, 128:144].reshape(128, 128)
        if nm == "wcs_h8":
            return wcs[8, 128:144]
        if nm in ("wlfe0", "wlfe1"):
            w9 = wlfe0 if nm == "wlfe0" else wlfe1
            v = np.zeros((128, 1152), np.float32)
            for t in range(9):
                v[:, t * 128:t * 128 + 128] = w9[t]
            return v
        if nm == "W1bd":
            return W1bd
        if nm == "W2bd":
            return W2bd
        if nm == "W1m":
            return W1m
        if nm == "W2m":
            return W2m.reshape(128, 512)
        if nm == "Ws0piq":
            return Ws0pi[:, 0:128]
        if nm == "Ws0piv":
            return Ws0pi[:, 128:256]
        if nm == "Ws0po":
            return ii["s0_po_w"].T
        if nm == "Ws1pi":
            return Ws1pi
        if nm == "Ws1po":
            return ii["s1_po_w"].T
        if nm == "P_a":
            return P_a
        if nm == "P_b":
            return P_a.T
        if nm == "P_a":
            return P_a
        if nm == "W1bd":
            return W1bd
        if nm == "W2bd":
            return W2bd
        if nm == "W1m":
            return W1m
        if nm == "W2m":
            return W2m.reshape(128, 512)
        if nm == "sel8":
            v = np.zeros((8, 1024), np.float32)
            for j in range(8):
                v[j, j * 128:(j + 1) * 128] = 1.0
            return v
        if nm == "meanmat":
            return np.full((128, 128), 1.0 / 128.0, np.float32)
        if nm == "ones8r":
            return np.ones((8, 128), np.float32)
        if nm == "ident":
            return np.eye(128, dtype=np.float32)
        raise KeyError(nm)

    pk16 = np.zeros((128, _N16), np.float32)
    for nm, (c0, w) in _C16.items():
        v = blk16(nm)
        pk16[0:v.shape[0], c0:c0 + w] = v

    rblk = {
        "meanmat": np.full((128, 128), 1.0 / 128.0, np.float32),
    }
    pkr = np.zeros((128, _NR), np.float32)
    for nm, (c0, w) in _CR.items():
        v = rblk[nm]
        pkr[0:v.shape[0], c0:c0 + w] = v

    s32 = {
        "mask1": mask1, "mask2": mask2,
        "fcw": np.ascontiguousarray(ii["fc_w"].T) / 1024.0,
        "b1m": b1m,
        "lamv": np.stack([np.full(128, lam, np.float32),
                          np.full(128, 1 - lam, np.float32)], 1),
        "gssfe_lo": ii["ssfe_g"][0:128], "bssfe_lo": ii["ssfe_b"][0:128],
        "gssfe_hi": ii["ssfe_g"][128:144], "bssfe_hi": ii["ssfe_b"][128:144],
        "gcc": ii["cc_g"], "bcc": ii["cc_b"],
        "gcs": ii["cs_g"], "bcs": ii["cs_b"],
        "glfe0": ii["lfe0_g"], "blfe0": ii["lfe0_b"],
        "glfe1": ii["lfe1_g"], "blfe1": ii["lfe1_b"],
        "ln1g": ii["ln1_g"], "ln1b": ii["ln1_b"],
        "gbng": ii["gbn_g"], "gbnb": ii["gbn_b"],
        "b1v": ii["qkv_b"][256:384], "b2v": ii["qkv2_b"][256:384],
        "b2m": ii["mlp_b2"],
        "bs0piq": bs0pi[0:128], "bs0po": ii["s0_po_b"],
        "bs1po": ii["s1_po_b"],
        "epsb": np.full(128, EPS, np.float32),
    }
    pk32 = np.zeros((128, _N32), np.float32)
    for nm, (c0, w) in _C32.items():
        v = np.asarray(s32[nm], np.float32)
        if v.ndim == 1:
            v = v[:, None]
        pk32[0:v.shape[0], c0:c0 + w] = v

    bbr = np.concatenate([ii["qkv_b"][0:256], ii["qkv2_b"][0:256],
                          bs0pi[128:256], bs1pi]).reshape(1, 768)

    com = {
        "pk16": pk16.astype(bf),
        "pk32": pk32,
        "pkr": _rr(pkr),
        "bbr": np.ascontiguousarray(bbr, np.float32),
    }
    in_maps = []
    for c in range(NCORES):
        m = dict(com)
        m["xpad"] = xpf[c * BPC:(c + 1) * BPC].astype(bf)
        m["xpad9"] = xp9[c * BPC:(c + 1) * BPC].astype(bf)
        in_maps.append(m)
    return in_maps


def _run(inputs, trace=False):
    global _COMPILED
    if _COMPILED is None:
        _COMPILED = _build()
    in_maps = _prep(inputs)
    res = bass_utils.run_bass_kernel_spmd(
        _COMPILED, in_maps, core_ids=list(range(NCORES)), trace=trace)
    out = np.concatenate([r["out"] for r in res.results], 0)
    return out.astype(np.float32), res


def kernel(**inputs):
    out, _ = _run(inputs, trace=False)
    return out
